# revision 64
# baseline (speedup 1.0000x reference)
"""LocallyConnected1d Trainium2 kernel (8 NeuronCores, SPMD).

Problem (hardcoded): x [128, 64, 1028] f32, weight [1, 64, 64, 256, 8] f32,
out[b, c, o] = sum_{ci,k} x[b, ci, 4*o + k] * w[c, ci, o, k] / sqrt(64),
out shape [128, 64, 256] f32.  O=256, K=8, S=4.

Strategy (v6, ~10.2us vs the 13.0us v3 baseline):
  - Shard O (output positions) 8 ways: core r owns o in [32r, 32r+32).
    This is the traffic-optimal sharding: x and w are each read exactly
    once across the fleet -> per-core input DMA = 2.13 MB (fp8).
  - fp8 E3M4 for x and w.  f32 PSUM accumulation; /sqrt(64) folded into
    the host-side gather (divide by 8).
  - With k = 4*k_hi + k_lo and t = o + k_hi, the x-side operand
    G[(ci,klo), t, b] = x[b,ci,4t+klo] is a pure reshape of x.  Each
    output o accumulates 4 matmuls (2 k_hi x 2 ci-halves) into one
    [128 b-partitions, 64 co-cols] psum region; the G block is the
    STATIONARY operand so each matmul is charged only its 64 moving
    cols.
  - G+W interleaved per-t in ONE dram tensor; 8 chunked SP-HWDGE DMAs
    keep the DMA bus saturated from first byte to last.  SP is released
    from the entry barrier and the chunk buffers are RAW sbuf tensors
    (no pool alloc-boundary wait), so chunk 0's descriptor-gen starts at
    ~150ns and the first byte lands at the 150+625+650 HWDGE floor.
  - OUTPUT via ONE kv_writeback: all psum pieces are copied (f32->f16)
    into a single SBUF tile [128, 2048]; a kv_writeback with
    prepare_only=True generates its descriptors early (hand-rolled
    deferred-RAW: the copy deps are demoted to no-sync on the prep and
    attached as sync deps to the trigger_dma), so the launch path after
    the last copy is just a Pool SEQ trigger + a 9-descriptor transfer
    (~102ns modeled) instead of six HWDGE/SWDGE output DMAs.
  - Endgame split fine ([17,18] / [19,20] / [21,22] / [23] chunks,
    separate psum banks per 2-o piece) so only 6 matmuls and one
    [128,128] copy trail the final 512-col chunk, with the two trailing
    copies on different engines (ACT/DVE).
  - Startup: one EVENT_SEMAPHORE_RANGE_CLEAR wipes every dynamically
    allocated sem before any data wait is evaluated (protects the first
    run against residue from a previously loaded NEFF); the tile-exit
    dma_reset/sem_clear pair and its guard barrier are elided in return
    (~350ns off the epilogue).
  - ~2.6us of warm-up matmuls complete the PE p-state ramp (0.65 -> 1.2
    -> 2.4 GHz after 3us of continuous busy) before the real stream;
    they borrow piece-w0's psum bank (its real matmuls overwrite).
"""

import sys

for _p in ("/opt/trn_rl_repo",):
    if _p not in sys.path:
        sys.path.insert(0, _p)

import numpy as np
import ml_dtypes

B, CI, CO, O, K, S = 128, 64, 64, 256, 8, 4
L = 1028
N_CORES = 8
O_LOC = O // N_CORES          # 32 output positions per core
NT = O_LOC + 1                # 33 t-blocks per core (t = o + k_hi)
# col layout per t-block in the fused gw dram tensor / sbuf tiles:
#   [G h0 (128 b) | G h1 (128 b) | W khi0 (h*64+co) | W khi1 (h*64+co)]
TBLK = 512
# Input pipeline chunks as explicit t-lists (local t in [0, 33)), in
# DMA issue = arrival order.  Small chunks first so compute starts
# ~3.5us in; t=23 last so only o22/o23 matmuls trail the last DMA.
CHUNK_TS = [
    [32, 8, 0, 1, 2, 3, 4, 5, 6, 7],
    [16, 9, 10, 11, 12, 13, 14, 15],
    [24, 25, 26, 27, 28],
    [29, 30, 31],
    [17, 18],
    [19, 20],
    [21, 22],
    [23],
]
T_ORDER = [t for ts in CHUNK_TS for t in ts]


# Edge t-blocks carry only one weight sect: t=0 has no k_hi=1 consumer
# (o=-1) and t=32 no k_hi=0 consumer (o=32 belongs to the next core),
# so those 128-col sects are dropped from the layout entirely.
def _t_cols(t):
    return 384 if t in (0, NT - 1) else TBLK


T_POS = {}
CHUNK_COLS = []
for _ci, _ts in enumerate(CHUNK_TS):
    _off = 0
    for _t in _ts:
        T_POS[_t] = (_ci, _off)
        _off += _t_cols(_t)
    CHUNK_COLS.append(_off)
TOTAL_COLS = sum(CHUNK_COLS)

_prog_cache = {}


def _build_program():
    if "nc" in _prog_cache:
        return _prog_cache["nc"]
    import concourse.tile as tile
    from concourse import bacc, mybir

    e3 = mybir.dt.float8e3
    f16 = mybir.dt.float16
    bf16 = mybir.dt.bfloat16
    f32 = mybir.dt.float32
    i32 = mybir.dt.int32

    nc = bacc.Bacc("TRN2", target_bir_lowering=False, debug=False,
                   num_devices=N_CORES)

    # Re-home the 4 prologue const-tile memsets from Pool (95ns q7 launch
    # each, serialized -> Pool reaches the entry barrier last at ~530ns)
    # onto DVE/ACT, two each, so every engine arrives at the barrier early
    # and the first input DMA issues ~300ns sooner.
    # Release SP from the ENTRY barrier: its first duty is the chunk-0
    # input DMA, which touches nothing the barrier protects (no sems, no
    # const tiles, HWDGE only), so it may issue ~250ns before the other
    # engines sync up.  Keep the sem accounting run-repeatable: drop SP's
    # release wait AND its -1 consumption, and shrink the Pool-side grant
    # from +4 to +3.  (In-context SP DMAs stay gated by their own
    # framework semaphores.)
    _done_sp = _done_pool = False
    for _bb in nc.m.functions[0].blocks:
        for _ins in _bb.instructions:
            if type(_ins).__name__ != "InstEventSemaphore":
                continue
            _si = _ins.sync_info
            if (not _done_sp and str(_ins.engine).endswith("SP")
                    and any(_w.ant_name and "release" in _w.ant_name
                            for _w in _si.on_wait)):
                # Make the wait trivially true (>=0) rather than emptying
                # sync_info (walrus rejects / mislowers bare EventSemaphores)
                # and drop the token consumption to match the +3 grant.
                for _w in _si.on_wait:
                    _w.wait_value = 0
                _si.on_update = []
                _done_sp = True
            elif (not _done_pool and str(_ins.engine).endswith("Pool")
                    and any(_u.ant_name and "release" in _u.ant_name
                            and _u.update_value == 4
                            for _u in _si.on_update)):
                for _u in _si.on_update:
                    if _u.ant_name and "release" in _u.ant_name:
                        _u.update_value = 3
                _done_pool = True
        if _done_sp and _done_pool:
            break
    assert _done_sp and _done_pool, (_done_sp, _done_pool)

    gw = nc.dram_tensor("gw", [128, TOTAL_COLS], e3,
                        kind="ExternalInput").ap()
    # kv_writeback out layout: [batch=1, d_head_inner=128, d_head_outer=1,
    # n_ctx=2048]; row p holds [o_loc, co] f16 for b-partition p.
    out = nc.dram_tensor("out", [1, 128, 1, O_LOC * CO], f16,
                         kind="ExternalOutput").ap()

    # First-run hygiene: a prior NEFF with a different sem layout may leave
    # residue anywhere in the dynamic range, which lets waits pass spuriously
    # (observed: stale DMAHW sems -> matmuls on unlanded chunks -> NaN).
    # Range-clear every sem this program will allocate (the barrier sems
    # 151/152 are below the range and in use).  Runs on Pool by ~700ns,
    # long before the first data wait is evaluated (~4000ns).
    _clr_base = nc.alloc_semaphore("clr_base")
    _rng = range(_clr_base.num, nc._kernel_sem_range.stop)
    nc.gpsimd.sem_clear(_rng)

    # Chunk buffers as RAW sbuf tensors (not pool tiles): their DMAs then
    # carry no pool alloc-boundary wait, so chunk 0's descriptor-gen starts
    # as soon as SP's sequencer reaches it (~200ns) instead of ~800ns.
    # Consumer matmuls still get RAW sync deps on the DMAs (verified: the
    # tracker follows raw tensors by name).
    craw = [nc.alloc_sbuf_tensor(f"c{i}raw", [128, CHUNK_COLS[i]], e3)
            for i in range(len(CHUNK_TS))]

    # The tile-context exit emits a dma_reset+sem_clear pair plus a second
    # all-engine barrier (~350ns serialized into the epilogue).  Our startup
    # range-clear already guarantees clean sems for repeat runs, so skip the
    # exit cleanup and its guard barrier: no-op the clear and swallow the
    # one barrier call that immediately follows it.  Both wrappers are
    # restored right after the context exits.
    _orig_clear = nc.clear_and_free_semaphores
    _orig_barrier = nc.all_engine_barrier
    _skip_state = {"skip_next_barrier": False}

    def _clear_noop(sems):
        _skip_state["skip_next_barrier"] = True
        nc._state.prepend_free_semaphores(
            [s.num if hasattr(s, "num") else s for s in sems])

    def _barrier_maybe_skip():
        if _skip_state["skip_next_barrier"]:
            _skip_state["skip_next_barrier"] = False
            return None
        return _orig_barrier()

    nc.clear_and_free_semaphores = _clear_noop
    nc.all_engine_barrier = _barrier_maybe_skip

    with tile.TileContext(nc) as tc:
        with (
            tc.tile_pool(name="gw", bufs=1) as gwpool,
            tc.tile_pool(name="ps", bufs=2, space="PSUM") as pspool,
            tc.tile_pool(name="ob", bufs=1) as obpool,
        ):
            # ---- output staging tile + kv_writeback prep (no data deps:
            # descriptors encode addresses; ctx idxs read at prep time).
            obig = obpool.tile([128, 1, 1, O_LOC * CO], f16, tag="obig")
            ctx = obpool.tile([128, 1], i32, tag="ctx")
            nc.vector.memset(ctx[:], 0)

            # Pre-allocate all psum piece tiles (8 banks exactly; the
            # warm-up borrows w0's bank -- its real matmuls start=True
            # overwrite, and PE is in-order).
            psws = {}
            for name, ncol in (("w0", 512), ("w1", 512), ("w3a", 256),
                               ("w3b", 256), ("w2a1", 128), ("w2a2", 128),
                               ("w2b1", 128), ("w2b2", 128)):
                psws[name] = pspool.tile([128, ncol], f32, tag=f"ps_{name}",
                                         bufs=1, name=f"ps_{name}")

            # ---- PE warm-up: ~3.5us of dummy matmuls so the p-state
            # ramp completes before the real stream starts.
            wu = gwpool.tile([128, 256], bf16, tag="warm")
            nc.vector.memset(wu[:], 0.0)
            wps = psws["w0"]
            for _ in range(16):
                nc.tensor.matmul(wps[:64, :256], wu[:, :64], wu[:, :],
                                 start=True, stop=True)

            # ---- input DMAs: one per chunk, all on the SP (sync) HWDGE
            # queue.  SP is released from the entry barrier and the raw
            # chunk buffers carry no alloc-boundary wait, so chunk 0's gen
            # starts as soon as SP's sequencer reaches it.  Chain no-sync
            # edges pin the scheduler to the tuned bus order.
            import bass_rust as _bass_rust
            cts = craw
            pos = 0
            _prev_dma = None
            for idx, ts in enumerate(CHUNK_TS):
                d = nc.sync.dma_start(craw[idx][:, :],
                                      gw[:, pos:pos + CHUNK_COLS[idx]]).ins
                if _prev_dma is not None:
                    _bass_rust.add_dep_helper(d, _prev_dma, False,
                                              "bus issue order")
                _prev_dma = d
                pos += CHUNK_COLS[idx]

            def mm4(psw, q, o, rev=False):
                """All 4 accumulating matmuls for output o into psum
                region [0:128, 64q:64(q+1)].  G stationary (free in the
                cost model), 64-co weight block moving."""
                dst = psw[:, q * 64:(q + 1) * 64]
                khis = (1, 0) if rev else (0, 1)
                for n_khi, khi in enumerate(khis):
                    t = o + khi
                    ci, base = T_POS[t]
                    # edge blocks hold a single weight sect at +256
                    wo = 256 if t in (0, NT - 1) else 256 + khi * 128
                    for h in (0, 1):
                        g = cts[ci][:, base + h * 128:base + h * 128 + 128]
                        w = cts[ci][:, base + wo + h * 64:
                                    base + wo + h * 64 + 64]
                        nc.tensor.matmul(dst, g, w,
                                         start=(n_khi == 0 and h == 0),
                                         stop=(n_khi == 1 and h == 1))

            def mm2(psw, q, o, khi, start, stop):
                """One (o, khi) pair of accumulating matmuls."""
                dst = psw[:, q * 64:(q + 1) * 64]
                t = o + khi
                ci, base = T_POS[t]
                wo = 256 if t in (0, NT - 1) else 256 + khi * 128
                for h in (0, 1):
                    g = cts[ci][:, base + h * 128:base + h * 128 + 128]
                    w = cts[ci][:, base + wo + h * 64:
                                base + wo + h * 64 + 64]
                    nc.tensor.matmul(dst, g, w,
                                     start=(start and h == 0),
                                     stop=(stop and h == 1))

            # Pieces in chunk-arrival order.  The endgame (o16..o23) is
            # split fine so both copy engines drain the tail in parallel:
            # w2a's copy is halved across ACT+DVE, and only 8/4 matmuls
            # trail chunks 5/6.
            # (name, o_list, copy engine or None for custom)
            pieces = [
                ("w0", list(range(0, 8)), nc.scalar),
                ("w1", list(range(8, 16)), nc.scalar),
                ("w3a", list(range(24, 28)), nc.vector),
                ("w3b", list(range(28, 32)), nc.vector),
            ]
            copy_insts = []
            for name, olist, cpq in pieces:
                ncol = len(olist) * 64
                psw = psws[name]
                for o in olist:
                    mm4(psw, o - olist[0], o)
                c0 = olist[0] * 64
                dst = obig[:, 0, 0, c0:c0 + ncol]
                if cpq is nc.scalar:
                    copy_insts.append(nc.scalar.copy(dst, psw[:]).ins)
                else:
                    copy_insts.append(nc.vector.tensor_copy(dst, psw[:]).ins)

            # ---- endgame: o16-17 (chunk 4), o18-19 + o20 khi0 (chunk 5),
            # o20-21 rest (chunk 6: t21/t22), o22-o23 (t23 = final chunk).
            ps_a1 = psws["w2a1"]
            ps_a2 = psws["w2a2"]
            ps_b1 = psws["w2b1"]
            ps_b2 = psws["w2b2"]
            for o in (16, 17):
                mm4(ps_a1, o - 16, o)
            for o in (18, 19):
                mm4(ps_a2, o - 18, o)
            # o20 khi0 reads t20 (chunk 5): issue it before chunk 6 lands.
            mm2(ps_b1, 0, 20, 0, start=True, stop=False)
            # chunk 6 (t21, t22): o20 khi1, o21 both; plus o22 khi0.
            mm2(ps_b1, 0, 20, 1, start=False, stop=True)
            mm2(ps_b1, 1, 21, 0, start=True, stop=False)
            mm2(ps_b1, 1, 21, 1, start=False, stop=True)
            # contiguous accumulation groups per o (interleaving start/stop
            # groups within one psum bank miscomputes on HW): o22 khi0(t22)
            # then khi1(t23); o23 khi1(t24, early) then khi0(t23).  6
            # matmuls trail the final chunk.
            mm2(ps_b2, 0, 22, 0, start=True, stop=False)
            mm2(ps_b2, 0, 22, 1, start=False, stop=True)
            mm2(ps_b2, 1, 23, 1, start=True, stop=False)
            mm2(ps_b2, 1, 23, 0, start=False, stop=True)
            # copies: a1/a2 on DVE as their chunks land, b1 on ACT (free
            # since w1's copy), b2 on DVE -- the trailing pieces drain on
            # different engines in parallel.
            copy_insts.append(
                nc.vector.tensor_copy(obig[:, 0, 0, 1024:1152], ps_a1[:]).ins)
            copy_insts.append(
                nc.vector.tensor_copy(obig[:, 0, 0, 1152:1280], ps_a2[:]).ins)
            copy_insts.append(
                nc.scalar.copy(obig[:, 0, 0, 1280:1408], ps_b1[:]).ins)
            copy_insts.append(
                nc.vector.tensor_copy(obig[:, 0, 0, 1408:1536], ps_b2[:]).ins)

            # ---- output writeback: hand-rolled deferred-RAW (kv_writeback
            # preps don't get the scatter_add-style automatic deferral).
            # One writeback: splitting it across DMASW lanes costs more in
            # serialized per-lane epilogue waits (~50ns each) than the
            # smaller tail transfer saves.  The prep is emitted AFTER the
            # copies so no WAR-on-DMA edges land on the copies; its RAW
            # sync deps on the copies are demoted to no-sync (descriptor-gen
            # only encodes addresses) so it runs early on the otherwise-idle
            # Pool engine, and the real data edges go on the TRIGGER.  The
            # baked DMA-completion sem MUST be the framework's DMASW lane-0
            # sem: the epilogue drain expects +16 on that handle.
            dma_sem = tc.sems.swdge_block()[0]
            prep = nc.gpsimd.kv_writeback(out, obig[:], ctx[:],
                                          prepare_only=True,
                                          sem=dma_sem).ins
            trig = nc.gpsimd.trigger_dma(count=1).ins
            for ci_inst in copy_insts:
                prep.try_remove_dependency(ci_inst.name)
                _bass_rust.add_dep_helper(prep, ci_inst, False,
                                          "prep encodes addresses only")
                _bass_rust.add_dep_helper(trig, ci_inst, True,
                                          "writeback DMA reads obig")

    nc.clear_and_free_semaphores = _orig_clear
    nc.all_engine_barrier = _orig_barrier

    # The output-DMA fence is a single SP end-wait on DMASW0 (>=16); after
    # it fires, the exit barrier serializes another ~400ns (SP arrive ->
    # Pool gather -> release -> every engine's final drain).  The barrier
    # only synchronizes engine ENDINGS -- the runtime treats the NEFF as
    # done when all queues drain, and SP's stream still holds the fence --
    # so make the END-BB barrier waits trivially true (>=0).  Updates are
    # kept: the gather/release sems still net to zero every run.
    for _bb in nc.m.functions[0].blocks:
        if _bb.name == "main":
            continue
        for _ins in _bb.instructions:
            _si = _ins.sync_info
            if not _si:
                continue
            if (any(_w.ant_name and "barrier" in _w.ant_name
                    for _w in _si.on_wait)
                    or any(_u.ant_name and "barrier" in _u.ant_name
                           for _u in _si.on_update)):
                _si.on_wait = [_w for _w in _si.on_wait
                               if not (_w.ant_name
                                       and "barrier" in _w.ant_name)]
                _si.on_update = [_u for _u in _si.on_update
                                 if not (_u.ant_name
                                         and "barrier" in _u.ant_name)]

    # The end-of-kernel waits all live on one SP drain pre-compile and are
    # split into ~2-condition EventSemaphores (order-preserving) during
    # compile.  The DMASW0 output fence fires LAST at runtime, so move it
    # to the END of the condition list: the already-satisfied DMAHW waits
    # then process before the fence instead of adding 50ns each after it.
    for _bb in nc.m.functions[0].blocks:
        for _ins in _bb.instructions:
            _si = _ins.sync_info
            if not _si or len(_si.on_wait) < 3:
                continue
            _ws = list(_si.on_wait)
            _sw = [w for w in _ws if w.ant_name and "DMASW" in w.ant_name]
            if _sw:
                _si.on_wait = [w for w in _ws if w not in _sw] + _sw

    nc.compile()
    _prog_cache["nc"] = nc
    return nc


def _shard_inputs(x, weight):
    """Host-side quantize + relayout.  Returns in_maps for the 8 cores."""
    e3 = ml_dtypes.float8_e3m4
    x = np.asarray(x, np.float32)
    w0 = np.asarray(weight, np.float32)[0]          # [Co, Ci, O, K]
    x8 = x.astype(e3)                               # [B, Ci, L]
    w8 = w0.astype(e3)                              # quantize BEFORE any scale

    # G_view[t, h, row=(ci_loc*4+klo), b] = x8[b, 32h+ci_loc, 4t+klo]
    xr = x8.reshape(B, CI, L // 4, 4)               # [b, ci, t, klo]
    gv = xr.transpose(1, 3, 2, 0)                   # [ci, klo, t, b]
    gv = np.ascontiguousarray(gv).reshape(2, 32, 4, L // 4, B)
    gv = gv.transpose(3, 0, 1, 2, 4).reshape(L // 4, 2, 128, B)  # [t,h,row,b]

    # W block layout per t: sect2 = k_hi=0 weights of o=t, sect3 =
    # k_hi=1 weights of o=t-1; cols within a sect = h*64 + co.
    wq = w8.reshape(CO, 2, 32, O, 2, 4)             # [co, h, cil, o, khi, klo]
    M = wq.transpose(3, 4, 1, 2, 5, 0)              # [o, khi, h, cil, klo, co]
    M = np.ascontiguousarray(M).reshape(O, 2, 2, 128, CO)  # [o,khi,h,row,co]
    Wfull = np.zeros((L // 4, 2, 128, 128), e3)     # [t, khi, row, (h,co)]
    Wfull[0:O, 0] = M[:, 0].transpose(0, 2, 1, 3).reshape(O, 128, 128)
    Wfull[1:O + 1, 1] = M[:, 1].transpose(0, 2, 1, 3).reshape(O, 128, 128)

    in_maps = []
    for r in range(N_CORES):
        t0 = r * O_LOC
        gs = gv[t0:t0 + NT]                         # [33, 2, 128, 128]
        ws = Wfull[t0:t0 + NT]                      # [33, 2, 128, 128]
        cols = []
        for t in T_ORDER:
            cols.append(gs[t, 0])
            cols.append(gs[t, 1])
            if t != NT - 1:
                cols.append(ws[t, 0])               # k_hi=0 sect
            if t != 0:
                cols.append(ws[t, 1])               # k_hi=1 sect
        comb = np.concatenate(cols, axis=1)         # [128, TOTAL_COLS]
        in_maps.append({"gw": np.ascontiguousarray(comb)})
    return in_maps


def _gather(results):
    out_full = np.empty((B, CO, O), np.float32)
    for r in range(N_CORES):
        d = results[r]["out"]                       # [1, 128, 1, 2048] f16
        d = np.asarray(d).reshape(B, O_LOC, CO)     # [b, o_loc, co]
        d = d.transpose(0, 2, 1).astype(np.float32) / 8.0
        out_full[:, :, r * O_LOC:(r + 1) * O_LOC] = d
    return out_full


def kernel(x, weight):
    from concourse.bass_utils import run_bass_kernel_spmd
    nc = _build_program()
    in_maps = _shard_inputs(x, weight)
    res = run_bass_kernel_spmd(nc, in_maps, list(range(N_CORES)))
    return _gather(res.results)
